# revision 4
# baseline (speedup 1.0000x reference)
"""Causal self-attention on 8 Trainium2 NeuronCores — v6.

v6 changes vs v5:
 - The softmax normalization is restructured. v5 ran, per attention
   unit, a [1,1024] DVE reciprocal: a single-partition iterative-divide
   (~8 cyc/elem = ~8.5us) that sat on the DVE FIFO ahead of the next
   unit's tri-multiplies and held the po PSUM ring — measured as a
   ~3.5us/unit serializer. v6 copies each unit's denominator row into
   partition hp of a per-quarter [4,1024] collector (the copy is what
   frees the po slot), then runs ONE partition-parallel [4,1024]
   reciprocal + the broadcasts + AT multiplies as a quarter-end batch,
   just before that quarter's out-projection.

v5 changes vs v4:
 - PV lag raised 2 -> 4 (build param `lag`). Real HW streams bf16
   matmuls at ~2 cols/cycle — twice the cost-model rate — so the
   lag-2 pipeline left PE arriving at PV(k) ~0.3-0.6us before exp(k)
   finished, stalling the strict-FIFO PE queue at every k-block
   (~120-160us/body measured: attn-only 217us + gemm-only 40us vs
   full 375us). With lag-4 the exp has ~3us of PE-queue time to
   complete; ACT (softmax exp, ~146us/body) becomes the sole
   critical chain.
 - ptp pool bufs 6 -> lag+4 so pt tiles live through the deeper lag.

v4 changes vs v3:
 - drops the unused 8 MiB xf chain input (per-exec input bytes cost real
   wall time through the PJRT/axon path); the timing chain serializes on
   an 8-float token instead: tik[8] in -> tok[8] out (tok is DMA'd from
   the last output tile, so it completes only after the body).


Problem: x[4,2048,1024] f32; qkv = x@w_qkv+b_qkv; 16 heads x 64; causal
softmax attention; out proj w_out/b_out.

Sharding: batch(4) x head-half(2) -> 8 cores. Each core computes one batch
element and 8 heads end-to-end; host sums the two partial out-projections
per batch and adds b_out.

v3 changes vs v2:
 - x^T is prepared on the host (bf16 cast + transpose, same class of
   preprocessing as the host-side weight casts/slices): the kernel takes
   xt[C, T] bf16 and just DMAs quarter-slices into SBUF. This removes the
   on-device f32 load + DVE cast + DRAM round-trip + xbar-transpose chain
   and the 32 PE transposes of quarter 0 (PE -12us, DVE -18us, DMA -24MB
   per body).
 - Everything else keeps the v2 structure: lag-2 PV pipeline, one exp per
   head-pair k-block, interleave pacing of qkv/outproj between attention
   units.
"""

import sys

sys.path.insert(0, "/opt/trn_rl_repo")

import numpy as np

B, T, C = 4, 2048, 1024
H, DH = 16, 64
HPC = 8           # heads per core
DPC = HPC * DH    # 512 per-core q/k/v features
NCORES = 8

_CACHE = {}


def _build(nbody=1, skip_attn=False, skip_proj=False, exp_mode="normal",
           lag=4):
    """nbody > 1 replicates the whole computation nbody times in one NEFF
    (each body re-reads the same xt and rewrites the same y). The per-exec
    dispatch floor (~400us through the axon tunnel) cancels in the slope
    T(nbody) vs T(1), giving true per-body device time. The graded kernel
    uses nbody=1. skip_attn/skip_proj/exp_mode carve out phases for timing
    attribution (output is then wrong)."""
    import concourse.bacc as bacc
    import concourse.mybir as mybir
    import concourse.tile as tile

    F32 = mybir.dt.float32
    BF16 = mybir.dt.bfloat16
    Exp = mybir.ActivationFunctionType.Exp
    add_op = mybir.AluOpType.add
    mult_op = mybir.AluOpType.mult
    is_ge = mybir.AluOpType.is_ge

    nc = bacc.Bacc("TRN2", target_bir_lowering=False, debug=False,
                   num_devices=NCORES)

    xt = nc.dram_tensor("xt", [C, T], BF16, kind="ExternalInput").ap()
    # tik/tok: 32-byte chain token for timing harnesses. tok is written
    # from the last output tile, so a consumer of tok has waited for the
    # whole body; feeding tok into the next execution's tik serializes
    # executions with only 32 bytes of extra I/O.
    tik = nc.dram_tensor("tik", [8], BF16, kind="ExternalInput").ap()
    tok = nc.dram_tensor("tok", [8], BF16, kind="ExternalOutput").ap()
    wq = nc.dram_tensor("wq", [C, DPC], BF16, kind="ExternalInput").ap()
    wk = nc.dram_tensor("wk", [C, DPC], BF16, kind="ExternalInput").ap()
    wv = nc.dram_tensor("wv", [C, DPC], BF16, kind="ExternalInput").ap()
    bq = nc.dram_tensor("bq", [DPC], F32, kind="ExternalInput").ap()
    bk = nc.dram_tensor("bk", [DPC], F32, kind="ExternalInput").ap()
    bv = nc.dram_tensor("bv", [DPC], F32, kind="ExternalInput").ap()
    wo = nc.dram_tensor("wo", [DPC, C], BF16, kind="ExternalInput").ap()
    y = nc.dram_tensor("y", [T, C], BF16, kind="ExternalOutput").ap()

    NT = T // 128          # 16 t-tiles of 128
    NCC = C // 128         # 8 contraction chunks for qkv proj
    NDC = DPC // 128       # 4 d-chunks of per-core features
    NQC = T // 512         # 4 q-chunks of 512

    with tile.TileContext(nc) as tc:
        import contextlib
        with contextlib.ExitStack() as stk:
            singles = stk.enter_context(tc.tile_pool(name="singles", bufs=1))
            small = stk.enter_context(tc.tile_pool(name="small", bufs=3))
            ptp = stk.enter_context(tc.tile_pool(name="ptp", bufs=lag + 4))
            p1 = stk.enter_context(tc.tile_pool(name="p1", bufs=1))
            pnrm = stk.enter_context(tc.tile_pool(name="pnrm", bufs=3))
            ps_big = stk.enter_context(
                tc.tile_pool(name="ps_big", bufs=2, space="PSUM"))
            ps_w = stk.enter_context(
                tc.tile_pool(name="ps_w", bufs=2, space="PSUM"))
            ps_o = stk.enter_context(
                tc.tile_pool(name="ps_o", bufs=2, space="PSUM"))

            QT = singles.tile([128, NDC, T], BF16, tag="QT")
            KT = singles.tile([128, NDC, T], BF16, tag="KT")
            V = singles.tile([128, NT, HPC, DH + 1], BF16, tag="V")
            AT = singles.tile([128, NDC, T], BF16, tag="AT")
            # full x^T resident: [128, 8, 2048] bf16 = 32 KiB/partition
            xts = singles.tile([128, NCC, T], BF16, tag="xts")

            bq_sb = singles.tile([128, NDC], F32, tag="bq_sb")
            bk_sb = singles.tile([128, NDC], F32, tag="bk_sb")
            bv_sb = singles.tile([1, DPC], F32, tag="bv_sb")
            bvb = singles.tile([128, DPC], F32, tag="bvb")
            nc.scalar.dma_start(
                out=bq_sb, in_=bq.rearrange("(d p) -> p d", p=128))
            nc.scalar.dma_start(
                out=bk_sb, in_=bk.rearrange("(d p) -> p d", p=128))
            nc.scalar.dma_start(
                out=bv_sb, in_=bv.rearrange("(a b) -> a b", a=1))
            nc.gpsimd.partition_broadcast(bvb, bv_sb, channels=128)

            # ones columns of V_aug
            nc.vector.memset(V[:, :, :, DH:DH + 1], 1.0)
            if skip_attn:
                nc.vector.memset(AT, 0.0)     # timing-only build
            if skip_proj:
                nc.vector.memset(QT, 0.0)
                nc.vector.memset(KT, 0.0)
                nc.vector.memset(V[:, :, :, 0:DH], 0.0)

            # causal triangle for the diagonal 128-blocks:
            # tri[k, q] = 1 if q >= k else 0; applied to pt by a DVE
            # multiply AFTER the exp.
            tri = singles.tile([128, 128], BF16, tag="tri")
            nc.vector.memset(tri, 1.0)
            nc.gpsimd.affine_select(
                out=tri, in_=tri, compare_op=is_ge, fill=0.0,
                base=0, pattern=[[1, 128]], channel_multiplier=-1)

            # preload the exp table off the critical path
            warm = singles.tile([1, 16], F32, tag="warm")
            nc.vector.memset(warm, 0.0)
            nc.scalar.activation(out=warm, in_=warm, func=Exp, scale=1.0)

            pt_const = None
            if exp_mode == "none":
                pt_const = singles.tile([128, 2, 512], BF16, tag="pt_const")
                nc.vector.memset(pt_const, 0.5)

            # weights (pre-cast to bf16 on the host)
            wq_sb = p1.tile([128, NCC, DPC], BF16, tag="wq_sb")
            wk_sb = p1.tile([128, NCC, DPC], BF16, tag="wk_sb")
            wv_sb = p1.tile([128, NCC, DPC], BF16, tag="wv_sb")
            wo_sb = p1.tile([128, NDC, C], BF16, tag="wo_sb")

            def units_xload(tq, xt_src):
                """Quarter tq of x^T: one DMA [128, 8, 512] bf16 (1 MiB) on
                the sync queue."""
                t0 = tq * 512

                def f():
                    nc.sync.dma_start(
                        out=xts[:, :, t0:t0 + 512],
                        in_=xt_src.rearrange(
                            "(c p) t -> p c t", p=128)[:, :, t0:t0 + 512])
                return [f]

            def units_wload():
                units = []

                def wload(w_dram, w_bf, cc):
                    def f():
                        # scalar (ACT) queue is the second HWDGE: weight
                        # DMAs there don't head-of-line block the x
                        # pipeline on the sync queue.
                        nc.scalar.dma_start(
                            out=w_bf[:, cc, :],
                            in_=w_dram[cc * 128:(cc + 1) * 128, :])
                    return f

                def oload(dc):
                    def f():
                        nc.scalar.dma_start(
                            out=wo_sb[:, dc, :],
                            in_=wo[dc * 128:(dc + 1) * 128, :])
                    return f

                for w_dram, w_bf in ((wq, wq_sb), (wk, wk_sb), (wv, wv_sb)):
                    for cc in range(NCC):
                        units.append(wload(w_dram, w_bf, cc))
                for dc in range(NDC):
                    units.append(oload(dc))
                return units

            def units_qkv(tq):
                """Emission units for quarter tq of qkv matmuls."""
                t0 = tq * 512
                units = []

                def qk(w_bf, OUT, b_col, dc):
                    # split into two emission halves (same PSUM accumulate
                    # group) so attention/exp work interleaves at ~0.5us
                    # granularity in the PE queue instead of ~1us
                    psq_box = {}

                    def f1():
                        psq_box["t"] = ps_w.tile([128, 512], F32, tag="w",
                                                 name="psq")
                        for cc in range(NCC // 2):
                            nc.tensor.matmul(
                                psq_box["t"],
                                w_bf[:, cc, dc * 128:(dc + 1) * 128],
                                xts[:, cc, t0:t0 + 512],
                                start=(cc == 0), stop=False)

                    def f2():
                        psq = psq_box["t"]
                        for cc in range(NCC // 2, NCC):
                            nc.tensor.matmul(
                                psq,
                                w_bf[:, cc, dc * 128:(dc + 1) * 128],
                                xts[:, cc, t0:t0 + 512],
                                start=False, stop=(cc == NCC - 1))
                        nc.vector.tensor_scalar_add(
                            out=OUT[:, dc, t0:t0 + 512], in0=psq,
                            scalar1=b_col[:, dc:dc + 1])
                    return [f1, f2]

                def vproj(tt):
                    def f():
                        psv = ps_w.tile([128, 512], F32, tag="w")
                        for cc in range(NCC):
                            nc.tensor.matmul(
                                psv,
                                xts[:, cc, t0 + tt * 128:t0 + (tt + 1) * 128],
                                wv_sb[:, cc, :],
                                start=(cc == 0), stop=(cc == NCC - 1))
                        nc.vector.tensor_tensor(
                            out=V[:, tq * 4 + tt, :, 0:DH],
                            in0=psv.rearrange("p (h c) -> p h c", h=HPC),
                            in1=bvb.rearrange("p (h c) -> p h c", h=HPC),
                            op=add_op)
                    return f

                for dc in range(NDC):
                    units.extend(qk(wq_sb, QT, bq_sb, dc))
                    units.extend(qk(wk_sb, KT, bk_sb, dc))
                for tt in range(4):
                    units.append(vproj(tt))
                return units

            def units_attention(hp, qc, qnorm):
                """Emission units for one head-pair's attention q-chunk.

                Batch b == k-chunk ki covers both pars in one [128, 2, 512]
                PSUM region; unit b emits [PV of batch b-1][scores of b]
                [exp of b] so PE's FIFO never waits on ACT when pacing
                inserts filler between units."""
                q0 = qc * 512
                nkc = (qc + 1) * 4      # causal k-chunks of 128
                po = [ps_o.tile([128, 512], F32, tag="o", name=f"po{par}")
                      for par in range(2)]
                state = {"pend": []}    # [(ki, pt, off)] awaiting PV
                units = []

                def pv_emit(ki, pt, off):
                    for par in range(2):
                        h = hp * 2 + par
                        nc.tensor.matmul(
                            po[par][0:DH + 1, off:512],
                            V[:, ki, h, :],
                            pt[:, par, off:512],
                            start=(ki == 0), stop=(ki == nkc - 1),
                            skip_group_check=True)

                def batch(ki):
                    k0 = ki * 128
                    off = max(0, k0 - q0)

                    def f():
                        pss = ps_big.tile([128, 2, 512], F32, tag="s")
                        for par in range(2):
                            nc.tensor.matmul(
                                pss[:, par, off:512],
                                KT[par * 64:(par + 1) * 64, hp,
                                   k0:k0 + 128],
                                QT[par * 64:(par + 1) * 64, hp,
                                   q0 + off:q0 + 512],
                                start=True, stop=True)
                        if exp_mode == "none":
                            # timing probe: no exp at all (WRONG output)
                            state["pend"].append((ki, pt_const, off))
                            if len(state["pend"]) > lag:
                                pv_emit(*state["pend"].pop(0))
                            return
                        pt = ptp.tile([128, 2, 512], BF16, tag="pt")
                        nc.scalar.activation(
                            out=pt[:, :, off:512],
                            in_=pss[:, :, off:512],
                            func=Exp, scale=0.125)
                        if k0 >= q0:
                            # zero the upper-triangle of the diagonal block
                            for par in range(2):
                                nc.vector.tensor_tensor(
                                    out=pt[:, par, off:off + 128],
                                    in0=pt[:, par, off:off + 128],
                                    in1=tri, op=mult_op)
                        # PV lags TWO batches: when PE reaches a PV matmul
                        # its exp has long finished, so PE's strict FIFO
                        # never blocks on ACT and ACT streams exps
                        # back-to-back.
                        state["pend"].append((ki, pt, off))
                        if len(state["pend"]) > lag:
                            pv_emit(*state["pend"].pop(0))
                    return f

                def finish():
                    # trailing PVs, then evacuate po: A^T rows into asb,
                    # denominator rows into partition hp of the quarter's
                    # [4, 1024] collector (these copies free the po ring).
                    # The reciprocal + normalize run as a quarter-end
                    # batch (units_norm), off the per-unit path.
                    for args in state["pend"]:
                        pv_emit(*args)
                    state["pend"] = []
                    asb = pnrm.tile([128, 512], F32, tag="asb", bufs=6)
                    for par in range(2):
                        nc.vector.tensor_copy(
                            out=asb[par * 64:(par + 1) * 64, :],
                            in_=po[par][0:DH, :])
                        nc.vector.tensor_copy(
                            out=qnorm["dsb"][hp * 32:hp * 32 + 1,
                                             par * 512:(par + 1) * 512],
                            in_=po[par][DH:DH + 1, :])
                    qnorm["asb"][hp] = asb

                for ki in range(nkc):
                    units.append(batch(ki))
                units.append(finish)
                return units

            def make_qnorm():
                # denominator rows land on partitions {0,32,64,96} (DVE
                # partition offsets must be 32-aligned); the reciprocal
                # runs over all 128 partitions — unused lanes compute in
                # parallel at no extra cost and are never read. memset
                # keeps those lanes defined (1.0 -> recip 1.0).
                dsb = pnrm.tile([128, 1024], F32, tag="dsb",
                                bufs=2, name="dsb")
                nc.vector.memset(dsb, 1.0)
                return {"dsb": dsb, "asb": {}}

            def units_norm(qc, qnorm):
                """Quarter-end normalization: one partition-parallel
                [4,1024] reciprocal, then per head-pair a broadcast and
                two gpsimd multiplies into AT."""
                q0 = qc * 512

                def recip():
                    rec4 = pnrm.tile([128, 1024], F32, tag="rec4", bufs=2)
                    nc.vector.reciprocal(out=rec4, in_=qnorm["dsb"])
                    qnorm["rec4"] = rec4

                def norm_hp(hp):
                    def f():
                        asb = qnorm["asb"][hp]
                        bc = pnrm.tile([128, 1024], F32, tag="bc", bufs=2)
                        # stage this head-pair's reciprocals on partition 0
                        # (gpsimd broadcast replicates partition 0; a
                        # nonzero base partition is sim-OK but not trusted
                        # on the real ucode)
                        rcp0 = pnrm.tile([1, 1024], F32, tag="rcp0",
                                         bufs=2)
                        nc.vector.tensor_copy(
                            out=rcp0,
                            in_=qnorm["rec4"][hp * 32:hp * 32 + 1, :])
                        nc.gpsimd.partition_broadcast(
                            bc, rcp0, channels=128)
                        for par in range(2):
                            nc.gpsimd.tensor_tensor(
                                out=AT[par * 64:(par + 1) * 64, hp,
                                       q0:q0 + 512],
                                in0=asb[par * 64:(par + 1) * 64, :],
                                in1=bc[par * 64:(par + 1) * 64,
                                       par * 512:(par + 1) * 512],
                                op=mult_op)
                    return f

                return [recip] + [norm_hp(hp) for hp in range(NDC)]

            last_ysb = {}

            def units_outproj(tt, y_dst):
                def one(cc2):
                    def f():
                        py = ps_w.tile([128, 512], F32, tag="w")
                        for hp in range(NDC):
                            nc.tensor.matmul(
                                py,
                                AT[:, hp, tt * 128:(tt + 1) * 128],
                                wo_sb[:, hp, cc2 * 512:(cc2 + 1) * 512],
                                start=(hp == 0), stop=(hp == NDC - 1))
                        ysb = small.tile([128, 512], BF16, tag="ysb", bufs=3)
                        nc.vector.tensor_copy(out=ysb, in_=py)
                        nc.sync.dma_start(
                            out=y_dst[tt * 128:(tt + 1) * 128,
                                      cc2 * 512:(cc2 + 1) * 512],
                            in_=ysb)
                        last_ysb["t"] = ysb
                    return f
                return [one(0), one(1)]

            def interleave_emit(a_units, b_units):
                """Emit a_units (PE-heavy fillers) and b_units (ACT-gated
                attention) round-robin, pacing a to spread across b."""
                na, nb = len(a_units), len(b_units)
                ai = 0
                for i, u in enumerate(b_units):
                    u()
                    target = (i + 1) * na // nb
                    while ai < target:
                        a_units[ai]()
                        ai += 1
                while ai < na:
                    a_units[ai]()
                    ai += 1

            def emit_body(xt_src, y_dst, pre_units=()):
                if not skip_proj:
                    for u in units_xload(0, xt_src):
                        u()
                for u in pre_units:
                    u()
                for tq in range(NQC):
                    a_units = []
                    if not skip_proj:
                        a_units += units_qkv(tq)
                        if tq + 1 < NQC:
                            a_units += units_xload(tq + 1, xt_src)
                    b_units = []
                    if tq >= 1 and not skip_attn:
                        qn = make_qnorm()
                        for hp in range(NDC):
                            b_units += units_attention(hp, tq - 1, qn)
                        b_units += units_norm(tq - 1, qn)
                    if b_units:
                        interleave_emit(a_units, b_units)
                    else:
                        for u in a_units:
                            u()
                # tail: quarter-3 attention (ACT-bound) + remaining
                # out-proj as PE filler
                tail_attn = []
                if not skip_attn:
                    qn = make_qnorm()
                    for hp in range(NDC):
                        tail_attn += units_attention(hp, NQC - 1, qn)
                    tail_attn += units_norm(NQC - 1, qn)
                tail_proj = []
                if not skip_proj:
                    for tt in range(0, (NQC - 1) * 4):
                        tail_proj += units_outproj(tt, y_dst)
                interleave_emit(tail_proj, tail_attn)
                if not skip_proj:
                    for tt in range((NQC - 1) * 4, NQC * 4):
                        for u in units_outproj(tt, y_dst):
                            u()

            for i in range(nbody):
                emit_body(xt, y,
                          pre_units=units_wload() if i == 0 else ())
            if last_ysb:
                nc.sync.dma_start(
                    out=tok.rearrange("(a b) -> a b", a=1),
                    in_=last_ysb["t"][0:1, 0:8])
            else:
                # skip_proj build: no ysb exists; satisfy the output
                zz = small.tile([1, 8], BF16, tag="ysb")
                nc.vector.memset(zz, 0.0)
                nc.sync.dma_start(
                    out=tok.rearrange("(a b) -> a b", a=1), in_=zz)

    nc.compile()
    return nc


LAST_RESULTS = None


def make_in_maps(x, w_qkv, b_qkv, w_out):
    import ml_dtypes
    BF = ml_dtypes.bfloat16
    in_maps = []
    for core in range(NCORES):
        b = core // 2
        h0 = (core % 2) * HPC
        d0 = h0 * DH
        in_maps.append({
            "xt": np.ascontiguousarray(x[b].T.astype(BF)),
            "tik": np.zeros(8, BF),
            "wq": np.ascontiguousarray(w_qkv[:, d0:d0 + DPC]).astype(BF),
            "wk": np.ascontiguousarray(
                w_qkv[:, C + d0:C + d0 + DPC]).astype(BF),
            "wv": np.ascontiguousarray(
                w_qkv[:, 2 * C + d0:2 * C + d0 + DPC]).astype(BF),
            "bq": np.ascontiguousarray(b_qkv[d0:d0 + DPC]),
            "bk": np.ascontiguousarray(b_qkv[C + d0:C + d0 + DPC]),
            "bv": np.ascontiguousarray(b_qkv[2 * C + d0:2 * C + d0 + DPC]),
            "wo": np.ascontiguousarray(w_out[d0:d0 + DPC, :]).astype(BF),
        })
    return in_maps


def kernel(x, w_qkv, b_qkv, w_out, b_out):
    global LAST_RESULTS
    from concourse import bass_utils

    x = np.ascontiguousarray(np.asarray(x, dtype=np.float32))
    w_qkv = np.ascontiguousarray(np.asarray(w_qkv, dtype=np.float32))
    b_qkv = np.ascontiguousarray(np.asarray(b_qkv, dtype=np.float32))
    w_out = np.ascontiguousarray(np.asarray(w_out, dtype=np.float32))
    b_out = np.ascontiguousarray(np.asarray(b_out, dtype=np.float32))

    if "nc" not in _CACHE:
        _CACHE["nc"] = _build()
    nc = _CACHE["nc"]

    in_maps = make_in_maps(x, w_qkv, b_qkv, w_out)

    # the axon-tunneled device occasionally throws transient INTERNAL /
    # UNRECOVERABLE errors; retry a couple of times before giving up.
    import time as _time
    res = None
    for attempt in range(3):
        try:
            res = bass_utils.run_bass_kernel_spmd(
                nc, in_maps, core_ids=list(range(NCORES)))
            break
        except Exception:
            if attempt == 2:
                raise
            _time.sleep(5.0)
    LAST_RESULTS = res

    out = np.empty((B, T, C), dtype=np.float32)
    for b in range(B):
        out[b] = (res.results[2 * b]["y"].astype(np.float32)
                  + res.results[2 * b + 1]["y"].astype(np.float32)
                  + b_out)
    return out


# revision 5
# speedup vs baseline: 1.3205x; 1.3205x over previous
"""Causal self-attention on 8 Trainium2 NeuronCores — v6.

v6 changes vs v5:
 - The softmax normalization is restructured. v5 ran, per attention
   unit, a [1,1024] DVE reciprocal: a single-partition iterative-divide
   (~8 cyc/elem = ~8.5us) that sat on the DVE FIFO ahead of the next
   unit's tri-multiplies and held the po PSUM ring — measured as a
   ~3.5us/unit serializer. v6 copies each unit's denominator row into
   partition hp of a per-quarter [4,1024] collector (the copy is what
   frees the po slot), then runs ONE partition-parallel [4,1024]
   reciprocal + the broadcasts + AT multiplies as a quarter-end batch,
   just before that quarter's out-projection.

v5 changes vs v4:
 - PV lag raised 2 -> 4 (build param `lag`). Real HW streams bf16
   matmuls at ~2 cols/cycle — twice the cost-model rate — so the
   lag-2 pipeline left PE arriving at PV(k) ~0.3-0.6us before exp(k)
   finished, stalling the strict-FIFO PE queue at every k-block
   (~120-160us/body measured: attn-only 217us + gemm-only 40us vs
   full 375us). With lag-4 the exp has ~3us of PE-queue time to
   complete; ACT (softmax exp, ~146us/body) becomes the sole
   critical chain.
 - ptp pool bufs 6 -> lag+4 so pt tiles live through the deeper lag.

v4 changes vs v3:
 - drops the unused 8 MiB xf chain input (per-exec input bytes cost real
   wall time through the PJRT/axon path); the timing chain serializes on
   an 8-float token instead: tik[8] in -> tok[8] out (tok is DMA'd from
   the last output tile, so it completes only after the body).


Problem: x[4,2048,1024] f32; qkv = x@w_qkv+b_qkv; 16 heads x 64; causal
softmax attention; out proj w_out/b_out.

Sharding: batch(4) x head-half(2) -> 8 cores. Each core computes one batch
element and 8 heads end-to-end; host sums the two partial out-projections
per batch and adds b_out.

v3 changes vs v2:
 - x^T is prepared on the host (bf16 cast + transpose, same class of
   preprocessing as the host-side weight casts/slices): the kernel takes
   xt[C, T] bf16 and just DMAs quarter-slices into SBUF. This removes the
   on-device f32 load + DVE cast + DRAM round-trip + xbar-transpose chain
   and the 32 PE transposes of quarter 0 (PE -12us, DVE -18us, DMA -24MB
   per body).
 - Everything else keeps the v2 structure: lag-2 PV pipeline, one exp per
   head-pair k-block, interleave pacing of qkv/outproj between attention
   units.
"""

import sys

sys.path.insert(0, "/opt/trn_rl_repo")

import numpy as np

B, T, C = 4, 2048, 1024
H, DH = 16, 64
HPC = 8           # heads per core
DPC = HPC * DH    # 512 per-core q/k/v features
NCORES = 8

_CACHE = {}


def _build(nbody=1, skip_attn=False, skip_proj=False, exp_mode="normal",
           lag=4):
    """nbody > 1 replicates the whole computation nbody times in one NEFF
    (each body re-reads the same xt and rewrites the same y). The per-exec
    dispatch floor (~400us through the axon tunnel) cancels in the slope
    T(nbody) vs T(1), giving true per-body device time. The graded kernel
    uses nbody=1. skip_attn/skip_proj/exp_mode carve out phases for timing
    attribution (output is then wrong)."""
    import concourse.bacc as bacc
    import concourse.mybir as mybir
    import concourse.tile as tile

    F32 = mybir.dt.float32
    BF16 = mybir.dt.bfloat16
    Exp = mybir.ActivationFunctionType.Exp
    add_op = mybir.AluOpType.add
    mult_op = mybir.AluOpType.mult
    is_ge = mybir.AluOpType.is_ge

    nc = bacc.Bacc("TRN2", target_bir_lowering=False, debug=False,
                   num_devices=NCORES)

    xt = nc.dram_tensor("xt", [C, T], BF16, kind="ExternalInput").ap()
    # tik/tok: 32-byte chain token for timing harnesses. tok is written
    # from the last output tile, so a consumer of tok has waited for the
    # whole body; feeding tok into the next execution's tik serializes
    # executions with only 32 bytes of extra I/O.
    tik = nc.dram_tensor("tik", [8], BF16, kind="ExternalInput").ap()
    tok = nc.dram_tensor("tok", [8], BF16, kind="ExternalOutput").ap()
    wq = nc.dram_tensor("wq", [C, DPC], BF16, kind="ExternalInput").ap()
    wk = nc.dram_tensor("wk", [C, DPC], BF16, kind="ExternalInput").ap()
    wv = nc.dram_tensor("wv", [C, DPC], BF16, kind="ExternalInput").ap()
    bq = nc.dram_tensor("bq", [DPC], F32, kind="ExternalInput").ap()
    bk = nc.dram_tensor("bk", [DPC], F32, kind="ExternalInput").ap()
    bv = nc.dram_tensor("bv", [DPC], F32, kind="ExternalInput").ap()
    wo = nc.dram_tensor("wo", [DPC, C], BF16, kind="ExternalInput").ap()
    y = nc.dram_tensor("y", [T, C], BF16, kind="ExternalOutput").ap()

    NT = T // 128          # 16 t-tiles of 128
    NCC = C // 128         # 8 contraction chunks for qkv proj
    NDC = DPC // 128       # 4 d-chunks of per-core features
    NQC = T // 512         # 4 q-chunks of 512

    with tile.TileContext(nc) as tc:
        import contextlib
        with contextlib.ExitStack() as stk:
            singles = stk.enter_context(tc.tile_pool(name="singles", bufs=1))
            small = stk.enter_context(tc.tile_pool(name="small", bufs=3))
            ptp = stk.enter_context(tc.tile_pool(name="ptp", bufs=lag + 4))
            p1 = stk.enter_context(tc.tile_pool(name="p1", bufs=1))
            pnrm = stk.enter_context(tc.tile_pool(name="pnrm", bufs=3))
            ps_big = stk.enter_context(
                tc.tile_pool(name="ps_big", bufs=2, space="PSUM"))
            ps_w = stk.enter_context(
                tc.tile_pool(name="ps_w", bufs=2, space="PSUM"))
            ps_o = stk.enter_context(
                tc.tile_pool(name="ps_o", bufs=2, space="PSUM"))

            QT = singles.tile([128, NDC, T], BF16, tag="QT")
            KT = singles.tile([128, NDC, T], BF16, tag="KT")
            V = singles.tile([128, NT, HPC, DH + 1], BF16, tag="V")
            AT = singles.tile([128, NDC, T], BF16, tag="AT")
            # full x^T resident: [128, 8, 2048] bf16 = 32 KiB/partition
            xts = singles.tile([128, NCC, T], BF16, tag="xts")

            bq_sb = singles.tile([128, NDC], F32, tag="bq_sb")
            bk_sb = singles.tile([128, NDC], F32, tag="bk_sb")
            bv_sb = singles.tile([1, DPC], F32, tag="bv_sb")
            bvb = singles.tile([128, DPC], F32, tag="bvb")
            nc.scalar.dma_start(
                out=bq_sb, in_=bq.rearrange("(d p) -> p d", p=128))
            nc.scalar.dma_start(
                out=bk_sb, in_=bk.rearrange("(d p) -> p d", p=128))
            nc.scalar.dma_start(
                out=bv_sb, in_=bv.rearrange("(a b) -> a b", a=1))
            nc.gpsimd.partition_broadcast(bvb, bv_sb, channels=128)

            # ones columns of V_aug
            nc.vector.memset(V[:, :, :, DH:DH + 1], 1.0)
            if skip_attn:
                nc.vector.memset(AT, 0.0)     # timing-only build
            if skip_proj:
                nc.vector.memset(QT, 0.0)
                nc.vector.memset(KT, 0.0)
                nc.vector.memset(V[:, :, :, 0:DH], 0.0)

            # causal triangle for the diagonal 128-blocks:
            # tri[k, q] = 1 if q >= k else 0; applied to pt by a DVE
            # multiply AFTER the exp.
            tri = singles.tile([128, 128], BF16, tag="tri")
            nc.vector.memset(tri, 1.0)
            nc.gpsimd.affine_select(
                out=tri, in_=tri, compare_op=is_ge, fill=0.0,
                base=0, pattern=[[1, 128]], channel_multiplier=-1)

            # preload the exp table off the critical path
            warm = singles.tile([1, 16], F32, tag="warm")
            nc.vector.memset(warm, 0.0)
            nc.scalar.activation(out=warm, in_=warm, func=Exp, scale=1.0)

            pt_const = None
            if exp_mode == "none":
                pt_const = singles.tile([128, 2, 512], BF16, tag="pt_const")
                nc.vector.memset(pt_const, 0.5)

            # weights (pre-cast to bf16 on the host)
            wq_sb = p1.tile([128, NCC, DPC], BF16, tag="wq_sb")
            wk_sb = p1.tile([128, NCC, DPC], BF16, tag="wk_sb")
            wv_sb = p1.tile([128, NCC, DPC], BF16, tag="wv_sb")
            wo_sb = p1.tile([128, NDC, C], BF16, tag="wo_sb")

            def units_xload(tq, xt_src):
                """Quarter tq of x^T: one DMA [128, 8, 512] bf16 (1 MiB) on
                the sync queue."""
                t0 = tq * 512

                def f():
                    nc.sync.dma_start(
                        out=xts[:, :, t0:t0 + 512],
                        in_=xt_src.rearrange(
                            "(c p) t -> p c t", p=128)[:, :, t0:t0 + 512])
                return [f]

            def units_wload():
                units = []

                def wload(w_dram, w_bf, cc):
                    def f():
                        # scalar (ACT) queue is the second HWDGE: weight
                        # DMAs there don't head-of-line block the x
                        # pipeline on the sync queue.
                        nc.scalar.dma_start(
                            out=w_bf[:, cc, :],
                            in_=w_dram[cc * 128:(cc + 1) * 128, :])
                    return f

                def oload(dc):
                    def f():
                        nc.scalar.dma_start(
                            out=wo_sb[:, dc, :],
                            in_=wo[dc * 128:(dc + 1) * 128, :])
                    return f

                for w_dram, w_bf in ((wq, wq_sb), (wk, wk_sb), (wv, wv_sb)):
                    for cc in range(NCC):
                        units.append(wload(w_dram, w_bf, cc))
                for dc in range(NDC):
                    units.append(oload(dc))
                return units

            def units_qkv(tq):
                """Emission units for quarter tq of qkv matmuls."""
                t0 = tq * 512
                units = []

                def qk(w_bf, OUT, b_col, dc):
                    # split into two emission halves (same PSUM accumulate
                    # group) so attention/exp work interleaves at ~0.5us
                    # granularity in the PE queue instead of ~1us
                    psq_box = {}

                    def f1():
                        psq_box["t"] = ps_w.tile([128, 512], F32, tag="w",
                                                 name="psq")
                        for cc in range(NCC // 2):
                            nc.tensor.matmul(
                                psq_box["t"],
                                w_bf[:, cc, dc * 128:(dc + 1) * 128],
                                xts[:, cc, t0:t0 + 512],
                                start=(cc == 0), stop=False)

                    def f2():
                        psq = psq_box["t"]
                        for cc in range(NCC // 2, NCC):
                            nc.tensor.matmul(
                                psq,
                                w_bf[:, cc, dc * 128:(dc + 1) * 128],
                                xts[:, cc, t0:t0 + 512],
                                start=False, stop=(cc == NCC - 1))
                        nc.vector.tensor_scalar_add(
                            out=OUT[:, dc, t0:t0 + 512], in0=psq,
                            scalar1=b_col[:, dc:dc + 1])
                    return [f1, f2]

                def vproj(tt):
                    def f():
                        psv = ps_w.tile([128, 512], F32, tag="w")
                        for cc in range(NCC):
                            nc.tensor.matmul(
                                psv,
                                xts[:, cc, t0 + tt * 128:t0 + (tt + 1) * 128],
                                wv_sb[:, cc, :],
                                start=(cc == 0), stop=(cc == NCC - 1))
                        nc.vector.tensor_tensor(
                            out=V[:, tq * 4 + tt, :, 0:DH],
                            in0=psv.rearrange("p (h c) -> p h c", h=HPC),
                            in1=bvb.rearrange("p (h c) -> p h c", h=HPC),
                            op=add_op)
                    return f

                for dc in range(NDC):
                    units.extend(qk(wq_sb, QT, bq_sb, dc))
                    units.extend(qk(wk_sb, KT, bk_sb, dc))
                for tt in range(4):
                    units.append(vproj(tt))
                return units

            def units_attention(hp, qc, qnorm):
                """Emission units for one head-pair's attention q-chunk.

                Batch b == k-chunk ki covers both pars in one [128, 2, 512]
                PSUM region; unit b emits [PV of batch b-1][scores of b]
                [exp of b] so PE's FIFO never waits on ACT when pacing
                inserts filler between units."""
                q0 = qc * 512
                nkc = (qc + 1) * 4      # causal k-chunks of 128
                po = [ps_o.tile([128, 512], F32, tag="o", name=f"po{par}")
                      for par in range(2)]
                state = {"pend": []}    # [(ki, pt, off)] awaiting PV
                units = []

                def pv_emit(ki, pt, off):
                    for par in range(2):
                        h = hp * 2 + par
                        nc.tensor.matmul(
                            po[par][0:DH + 1, off:512],
                            V[:, ki, h, :],
                            pt[:, par, off:512],
                            start=(ki == 0), stop=(ki == nkc - 1),
                            skip_group_check=True)

                def batch(ki):
                    k0 = ki * 128
                    off = max(0, k0 - q0)

                    def f():
                        pss = ps_big.tile([128, 2, 512], F32, tag="s")
                        for par in range(2):
                            nc.tensor.matmul(
                                pss[:, par, off:512],
                                KT[par * 64:(par + 1) * 64, hp,
                                   k0:k0 + 128],
                                QT[par * 64:(par + 1) * 64, hp,
                                   q0 + off:q0 + 512],
                                start=True, stop=True)
                        if exp_mode == "none":
                            # timing probe: no exp at all (WRONG output)
                            state["pend"].append((ki, pt_const, off))
                            if len(state["pend"]) > lag:
                                pv_emit(*state["pend"].pop(0))
                            return
                        pt = ptp.tile([128, 2, 512], BF16, tag="pt")
                        nc.scalar.activation(
                            out=pt[:, :, off:512],
                            in_=pss[:, :, off:512],
                            func=Exp, scale=0.125)
                        if k0 >= q0:
                            # zero the upper-triangle of the diagonal block
                            for par in range(2):
                                nc.vector.tensor_tensor(
                                    out=pt[:, par, off:off + 128],
                                    in0=pt[:, par, off:off + 128],
                                    in1=tri, op=mult_op)
                        # PV lags TWO batches: when PE reaches a PV matmul
                        # its exp has long finished, so PE's strict FIFO
                        # never blocks on ACT and ACT streams exps
                        # back-to-back.
                        state["pend"].append((ki, pt, off))
                        if len(state["pend"]) > lag:
                            pv_emit(*state["pend"].pop(0))
                    return f

                def finish():
                    # trailing PVs, then evacuate po: A^T rows into asb,
                    # denominator rows into partition hp of the quarter's
                    # [4, 1024] collector (these copies free the po ring).
                    # The reciprocal + normalize run as a quarter-end
                    # batch (units_norm), off the per-unit path.
                    for args in state["pend"]:
                        pv_emit(*args)
                    state["pend"] = []
                    asb = pnrm.tile([128, 512], F32, tag="asb", bufs=6)
                    for par in range(2):
                        nc.vector.tensor_copy(
                            out=asb[par * 64:(par + 1) * 64, :],
                            in_=po[par][0:DH, :])
                        nc.vector.tensor_copy(
                            out=qnorm["dsb"][hp * 32:hp * 32 + 1,
                                             par * 512:(par + 1) * 512],
                            in_=po[par][DH:DH + 1, :])
                    qnorm["asb"][hp] = asb

                for ki in range(nkc):
                    units.append(batch(ki))
                units.append(finish)
                return units

            def make_qnorm():
                # denominator rows land on partitions {0,32,64,96} (DVE
                # partition offsets must be 32-aligned); the reciprocal
                # runs over all 128 partitions — unused lanes compute in
                # parallel at no extra cost and are never read. memset
                # keeps those lanes defined (1.0 -> recip 1.0).
                dsb = pnrm.tile([128, 1024], F32, tag="dsb",
                                bufs=2, name="dsb")
                nc.vector.memset(dsb, 1.0)
                return {"dsb": dsb, "asb": {}}

            def units_norm(qc, qnorm):
                """Quarter-end normalization: one partition-parallel
                [4,1024] reciprocal, then per head-pair a broadcast and
                two gpsimd multiplies into AT."""
                q0 = qc * 512

                def recip():
                    rec4 = pnrm.tile([128, 1024], F32, tag="rec4", bufs=2)
                    nc.vector.reciprocal(out=rec4, in_=qnorm["dsb"])
                    qnorm["rec4"] = rec4

                def norm_hp(hp):
                    def f():
                        asb = qnorm["asb"][hp]
                        bc = pnrm.tile([128, 1024], F32, tag="bc", bufs=2)
                        # stage this head-pair's reciprocals on partition 0
                        # (gpsimd broadcast replicates partition 0; a
                        # nonzero base partition is sim-OK but not trusted
                        # on the real ucode)
                        rcp0 = pnrm.tile([1, 1024], F32, tag="rcp0",
                                         bufs=2)
                        nc.vector.tensor_copy(
                            out=rcp0,
                            in_=qnorm["rec4"][hp * 32:hp * 32 + 1, :])
                        nc.gpsimd.partition_broadcast(
                            bc, rcp0, channels=128)
                        for par in range(2):
                            nc.gpsimd.tensor_tensor(
                                out=AT[par * 64:(par + 1) * 64, hp,
                                       q0:q0 + 512],
                                in0=asb[par * 64:(par + 1) * 64, :],
                                in1=bc[par * 64:(par + 1) * 64,
                                       par * 512:(par + 1) * 512],
                                op=mult_op)
                    return f

                return [recip] + [norm_hp(hp) for hp in range(NDC)]

            last_ysb = {}

            def units_outproj(tt, y_dst):
                def one(cc2):
                    def f():
                        py = ps_w.tile([128, 512], F32, tag="w")
                        for hp in range(NDC):
                            nc.tensor.matmul(
                                py,
                                AT[:, hp, tt * 128:(tt + 1) * 128],
                                wo_sb[:, hp, cc2 * 512:(cc2 + 1) * 512],
                                start=(hp == 0), stop=(hp == NDC - 1))
                        ysb = small.tile([128, 512], BF16, tag="ysb", bufs=3)
                        nc.vector.tensor_copy(out=ysb, in_=py)
                        nc.sync.dma_start(
                            out=y_dst[tt * 128:(tt + 1) * 128,
                                      cc2 * 512:(cc2 + 1) * 512],
                            in_=ysb)
                        last_ysb["t"] = ysb
                    return f
                return [one(0), one(1)]

            def interleave_emit(a_units, b_units):
                """Emit a_units (PE-heavy fillers) and b_units (ACT-gated
                attention) round-robin, pacing a to spread across b."""
                na, nb = len(a_units), len(b_units)
                ai = 0
                for i, u in enumerate(b_units):
                    u()
                    target = (i + 1) * na // nb
                    while ai < target:
                        a_units[ai]()
                        ai += 1
                while ai < na:
                    a_units[ai]()
                    ai += 1

            def emit_body(xt_src, y_dst, pre_units=()):
                if not skip_proj:
                    for u in units_xload(0, xt_src):
                        u()
                for u in pre_units:
                    u()
                for tq in range(NQC):
                    a_units = []
                    if not skip_proj:
                        # prefetch next quarter's x^T FIRST so the 1 MiB
                        # DMA has the whole quarter to land before its
                        # consumers run
                        if tq + 1 < NQC:
                            a_units += units_xload(tq + 1, xt_src)
                        a_units += units_qkv(tq)
                    b_units = []
                    if tq >= 1 and not skip_attn:
                        qn = make_qnorm()
                        for hp in range(NDC):
                            b_units += units_attention(hp, tq - 1, qn)
                        b_units += units_norm(tq - 1, qn)
                    if b_units:
                        interleave_emit(a_units, b_units)
                    else:
                        for u in a_units:
                            u()
                # tail: quarter-3 attention (ACT-bound) + remaining
                # out-proj as PE filler
                tail_attn = []
                if not skip_attn:
                    qn = make_qnorm()
                    for hp in range(NDC):
                        tail_attn += units_attention(hp, NQC - 1, qn)
                    tail_attn += units_norm(NQC - 1, qn)
                tail_proj = []
                if not skip_proj:
                    for tt in range(0, (NQC - 1) * 4):
                        tail_proj += units_outproj(tt, y_dst)
                interleave_emit(tail_proj, tail_attn)
                if not skip_proj:
                    for tt in range((NQC - 1) * 4, NQC * 4):
                        for u in units_outproj(tt, y_dst):
                            u()

            for i in range(nbody):
                emit_body(xt, y,
                          pre_units=units_wload() if i == 0 else ())
            if last_ysb:
                nc.sync.dma_start(
                    out=tok.rearrange("(a b) -> a b", a=1),
                    in_=last_ysb["t"][0:1, 0:8])
            else:
                # skip_proj build: no ysb exists; satisfy the output
                zz = small.tile([1, 8], BF16, tag="ysb")
                nc.vector.memset(zz, 0.0)
                nc.sync.dma_start(
                    out=tok.rearrange("(a b) -> a b", a=1), in_=zz)

    nc.compile()
    return nc


LAST_RESULTS = None


def make_in_maps(x, w_qkv, b_qkv, w_out):
    import ml_dtypes
    BF = ml_dtypes.bfloat16
    in_maps = []
    for core in range(NCORES):
        b = core // 2
        h0 = (core % 2) * HPC
        d0 = h0 * DH
        in_maps.append({
            "xt": np.ascontiguousarray(x[b].T.astype(BF)),
            "tik": np.zeros(8, BF),
            "wq": np.ascontiguousarray(w_qkv[:, d0:d0 + DPC]).astype(BF),
            "wk": np.ascontiguousarray(
                w_qkv[:, C + d0:C + d0 + DPC]).astype(BF),
            "wv": np.ascontiguousarray(
                w_qkv[:, 2 * C + d0:2 * C + d0 + DPC]).astype(BF),
            "bq": np.ascontiguousarray(b_qkv[d0:d0 + DPC]),
            "bk": np.ascontiguousarray(b_qkv[C + d0:C + d0 + DPC]),
            "bv": np.ascontiguousarray(b_qkv[2 * C + d0:2 * C + d0 + DPC]),
            "wo": np.ascontiguousarray(w_out[d0:d0 + DPC, :]).astype(BF),
        })
    return in_maps


def kernel(x, w_qkv, b_qkv, w_out, b_out):
    global LAST_RESULTS
    from concourse import bass_utils

    x = np.ascontiguousarray(np.asarray(x, dtype=np.float32))
    w_qkv = np.ascontiguousarray(np.asarray(w_qkv, dtype=np.float32))
    b_qkv = np.ascontiguousarray(np.asarray(b_qkv, dtype=np.float32))
    w_out = np.ascontiguousarray(np.asarray(w_out, dtype=np.float32))
    b_out = np.ascontiguousarray(np.asarray(b_out, dtype=np.float32))

    if "nc" not in _CACHE:
        _CACHE["nc"] = _build()
    nc = _CACHE["nc"]

    in_maps = make_in_maps(x, w_qkv, b_qkv, w_out)

    # the axon-tunneled device occasionally throws transient INTERNAL /
    # UNRECOVERABLE errors; retry a couple of times before giving up.
    import time as _time
    res = None
    for attempt in range(3):
        try:
            res = bass_utils.run_bass_kernel_spmd(
                nc, in_maps, core_ids=list(range(NCORES)))
            break
        except Exception:
            if attempt == 2:
                raise
            _time.sleep(5.0)
    LAST_RESULTS = res

    out = np.empty((B, T, C), dtype=np.float32)
    for b in range(B):
        out[b] = (res.results[2 * b]["y"].astype(np.float32)
                  + res.results[2 * b + 1]["y"].astype(np.float32)
                  + b_out)
    return out
